# revision 16
# baseline (speedup 1.0000x reference)
"""Trainium2 Bass kernel for nn_AttentionTest_14044543058050.

Reference computation (B=4, S=8, N=1024, D=512, HEADS=4):
    for h in heads:
        qkv = selu(x @ Wqkv[h] + bqkv[h]);  q,k,v = split(qkv)
        att = softmax((q @ k.T / D) @ v, axis=-1)      # softmax over D!
        proj_h = gelu(att @ Wp[h] + bp[h])
    out = pose_encoding(proj_3 + 0.01 * proj_0)

Key algebraic facts exploited (same as the 433us baseline):
  * heads 1 and 2 are dead code -> only heads 0 and 3 computed.
  * (q k^T) v reassociates exactly to q (k^T v) -- no N x N scores.
  * softmax(L) @ Wp defers normalization past the Wp matmul.
  * selu decomposed as max(U,0) + min(64a e^{U/64} - 64a, 0) with the
    lam^3/D constant folded into the exp(kappa L) activation scale.

v4 performance restructure over the 433us/377us baseline:
  * Every matmul accumulation pair (k|v, q j0|j1, C c|c+1, L j0|j1,
    proj|rowsum) now shares ONE two-bank PSUM tile.  Both half-groups
    become schedulable at the same sim time, so the Tile scheduler keeps
    the interleaved emission order and the post-legalize LDWEIGHTS dedup
    actually fires (the baseline's per-bank tiles made the scheduler
    serialize whole accumulation groups, separating same-key LDWs: only
    ~40 of ~500 dedup opportunities hit; now they all do).
  * All A1/A2 pointwise ops run once per 1024-wide pair instead of twice
    per 512: ACT exp (+ csb evacuate) amortize the (352cyc) fixed
    overhead; DVE ts/stt halve their per-op overhead and sem traffic.
  * softmax rows sum to 1, so the proj bias folds into the proj weights
    (Wp' = Wp + bp): with deferred normalization pre = pp*rsr where
    pp = sum(elt * 64(Wp+bp)) equals att@Wp + bp exactly.  The B-stage
    scalar_tensor_tensor disappears; pre is an ACT Copy with the
    per-partition 1/rowsum as the scale port.
  * GPSIMD runs NOTHING: its SBUF port is shared with the DVE, and
    hardware-measured here its tensor_tensor traffic degrades concurrent
    DVE tensor_scalar/tensor_tensor from their 2x/4x packed modes to ~1x
    (427ns -> 785ns, 409ns -> 1331ns).  The whole gelu/combine chain
    moved to the DVE, which is a net win once its fast modes survive.
  * A slice of the k/v selu tiles evacuate max(U,0) on the ACT (Relu,
    same table set as Exp) so the DVE does an SBUF-only tensor_tensor
    instead of the 1x-rate PSUM-read scalar_tensor_tensor; the count is
    chosen to balance ACT vs DVE at ~215us each.
  * Output, pose table and the epsilon-combine chain run in bf16; the
    f32 upcast happens on the host after the gather (DMA volume halves).
  The point of all of it: the PE (the roofline engine at ~200us of
  fp8-DoubleRow matmul stream) stops starving on the DVE semaphore and
  stops re-throttling to K=4/8 (the baseline oscillated HAM 7 times,
  ~110us at half clock).

Sharding: the 32 (b, s) pairs are split 4-per-core across 8 NeuronCores;
weights are replicated.  Both live heads of a pair stay on one core.

Precision: all matmuls fp8e4m3 DoubleRow (weights pre-scaled by 64,
scales cancel inside exp(kappa L) as in the baseline).  bf16 output
rounding adds ~4e-3 of output absmax; total measured error stays well
inside the 2e-2 gate.
"""

import math
from contextlib import ExitStack

import numpy as np
import ml_dtypes

import concourse.bass as bass
import concourse.tile as tile
import concourse.mybir as mybir
from concourse.vector_clock import ScopedClock
from concourse.bass_utils import run_bass_kernel_spmd

B, S, N, D = 4, 8, 1024, 512
HEADS_USED = (0, 3)
EPS = 0.01
LAM = 1.0507009873554805
ALPHA = 1.6732632423543772
LN_ALPHA = math.log(ALPHA)
KAPPA = LAM ** 3 / D
NCORES = 8
PAIRS = (B * S) // NCORES  # 4 (b,s) pairs per core

bf16 = mybir.dt.bfloat16
f32 = mybir.dt.float32
fp8 = mybir.dt.float8e4
DR = mybir.MatmulPerfMode.DoubleRow
WSCALE = 64.0
CSC = 2048.0  # C-cast divisor: keeps |csb| < fp8e4m3 max 240
AF = mybir.ActivationFunctionType
ALU = mybir.AluOpType
P = 128
DC = D // P   # 4 chunks of 128 along D
NC_ = N // P  # 8 chunks of 128 along N
GC = 0.3989422804014327  # gelu(x) ~ x*(GC*x + 0.5), exact to <2e-6 here
# ACT-relu offload of k/v selu tiles: measured a wash (the fp8-output
# tensor_tensor runs at 1x, same as the PSUM-read stt it replaces) while
# costing an extra ACT pass -- disabled.
KV_RELU_SWAP_T = ()


class _SplitDrainTileContext(tile.TileContext):
    """TileContext adapted to this container's walrus build, which rejects
    more than ONE sync-wait command per instruction (any format).  After
    Tile assigns semaphores we hoist every extra wait onto a same-engine
    NoOp inserted right before the instruction (engine queues are in-order,
    so waiting earlier on the same queue is equivalent), and the final
    drain's aggregated wait list is split the same way.

    Additionally dedupes LDWEIGHTS: tile_legalize splits every non-f32
    matmul into InstLdweights + non-self-loading InstMatmult, reloading
    the stationary operand even when consecutive matmuls share it.  The
    PE serializes LDWEIGHTS with the matmul stream (~140 ns each), so a
    repeated load of the identical weights AP is pure loss.  We replace
    any InstLdweights whose (offset, access pattern, dtype, perf mode)
    equals the previous weight load on the PE queue -- with no other
    weight-mutating PE instruction in between -- by a NoOp carrying the
    same sync_info (waits and sem updates preserved)."""

    def _dedupe_ldweights(self):
        nc = self.nc
        pe = mybir.EngineType.PE
        deduped = 0
        for f in nc.m.functions:
            for bb in f.blocks:
                last_key = None
                newl = []
                for inst in bb.instructions:
                    tn = type(inst).__name__
                    if tn == "InstLdweights":
                        key = None
                        try:
                            arg = inst.ins[0]
                            key = (
                                arg.offset,
                                tuple(tuple(p) for p in arg.ap),
                                str(arg.dtype),
                                str(inst.perf_mode),
                                inst.is_transpose,
                                inst.tile_size,
                                inst.tile_position,
                            )
                        except Exception:
                            key = None
                        if key is not None and key == last_key:
                            nop = mybir.InstNoOp(
                                name=nc.get_next_instruction_name(),
                                ins=[], outs=[],
                            )
                            nop.engine = inst.engine
                            nop.sync_info = inst.sync_info
                            nc.register_instruction(nop)
                            newl.append(nop)
                            deduped += 1
                            continue
                        last_key = key
                    elif inst.engine == pe:
                        if tn == "InstMatmult":
                            if inst.ldweights is not False:
                                # self-loading matmul clobbers the array
                                last_key = None
                        elif tn not in (
                            "InstNoOp", "InstEventSemaphore", "InstDrain"
                        ):
                            last_key = None
                    newl.append(inst)
                bb.instructions = newl
        self._ldw_deduped = deduped

    def _hoist_extra_waits(self):
        nc = self.nc
        for f in nc.m.functions:
            for bb in f.blocks:
                insts = bb.instructions
                if not any(
                    i.sync_info and i.sync_info.on_wait and len(i.sync_info.on_wait) > 1
                    for i in insts
                ):
                    continue
                newl = []
                for inst in insts:
                    si = inst.sync_info
                    if si and si.on_wait and len(si.on_wait) > 1:
                        waits = list(si.on_wait)
                        for w in waits[:-1]:
                            nop = mybir.InstNoOp(
                                name=nc.get_next_instruction_name(), ins=[], outs=[]
                            )
                            nop.engine = inst.engine
                            nop.sync_info = mybir.SyncInfo(
                                on_wait=[w], on_update=[]
                            )
                            nc.register_instruction(nop)
                            newl.append(nop)
                        si.on_wait = [waits[-1]]
                    newl.append(inst)
                bb.instructions = newl

    def _drain_and_barrier(self, tick_clock, wait_clock):
        nc = self.nc
        self._dedupe_ldweights()
        self._hoist_extra_waits()
        nop0 = nc.sync.nop(nofuse=True)
        wait_clock.add_sem_waits(
            nop0.ins, ScopedClock({None: tick_clock.global_clock})
        )
        si = nop0.ins.sync_info
        waits = list(si.on_wait) if si is not None and si.on_wait else []
        if len(waits) > 1:
            si.on_wait = waits[:1]
            for w in waits[1:]:
                nop = nc.sync.nop(nofuse=True)
                nsi = nop.ins.sync_info
                if nsi is None:
                    nop.ins.sync_info = mybir.SyncInfo(on_wait=[w], on_update=[])
                else:
                    nsi.on_wait = [w]
        nc.sync.drain()
        nc.all_engine_barrier()
        assert self.sems is not None
        popped = nc._tile_sem_poison_stack.pop()
        assert popped is self._sem_poison
        nc.clear_and_free_semaphores(list(self.sems.allocated().values()))
        nc.all_engine_barrier()


def build_program(n_pairs=PAIRS):
    nc = bass.Bass()

    xT_d = nc.dram_tensor("xT", [n_pairs, D, N], fp8, kind="ExternalInput")
    wq_d = nc.dram_tensor("wq", [2, D, D], fp8, kind="ExternalInput")
    wk_d = nc.dram_tensor("wk", [2, D, D], fp8, kind="ExternalInput")
    wv_d = nc.dram_tensor("wv", [2, D, D], fp8, kind="ExternalInput")
    wp_d = nc.dram_tensor("wp", [2, D, D], fp8, kind="ExternalInput")
    bkvr_d = nc.dram_tensor("bkvr8", [2, 1, 2, 2 * D], fp8, kind="ExternalInput")
    bqe_d = nc.dram_tensor("bqe", [2, P, DC], f32, kind="ExternalInput")
    bqs_d = nc.dram_tensor("bqs", [2, P, 3, DC], f32, kind="ExternalInput")
    pe_d = nc.dram_tensor("pe", [N, D], bf16, kind="ExternalInput")
    out_d = nc.dram_tensor("out", [n_pairs, N, D], bf16, kind="ExternalOutput")

    with _SplitDrainTileContext(nc) as tc, ExitStack() as ctx:
        xpool = ctx.enter_context(tc.tile_pool(name="xt", bufs=2))
        qtpool = ctx.enter_context(tc.tile_pool(name="qt", bufs=2))
        kvpool = ctx.enter_context(tc.tile_pool(name="kv", bufs=2))
        cpool = ctx.enter_context(tc.tile_pool(name="csb", bufs=2))
        eltpool = ctx.enter_context(tc.tile_pool(name="elt", bufs=3))
        zpool = ctx.enter_context(tc.tile_pool(name="zt", bufs=2))
        opool = ctx.enter_context(tc.tile_pool(name="osb", bufs=2))
        rsrpool = ctx.enter_context(tc.tile_pool(name="rsr", bufs=3))
        tb = ctx.enter_context(tc.tile_pool(name="tb", bufs=8))
        tf = ctx.enter_context(tc.tile_pool(name="tf", bufs=8))
        # single two-bank-per-tile PSUM pool: every accumulation pair
        # (k|v, q j0|j1, C c|c+1, L j0|j1, proj|rowsum) lives in one tile
        # so both half-groups allocate (and become schedulable) together.
        ps2 = ctx.enter_context(tc.tile_pool(name="ps2", bufs=4, space="PSUM"))

        xt0 = xpool.tile([P, DC, N], fp8, tag="xt", name="xt_pre0")
        nc.sync.dma_start(xt0[:], xT_d[0].rearrange("(c q) n -> q c n", q=P))

        wpool = ctx.enter_context(tc.tile_pool(name="warm", bufs=1))
        warm = wpool.tile([P, 512], bf16, tag="warm")
        nc.vector.memset(warm[:], 0.0)
        wps = ps2.tile([P, 2 * D], f32, tag="ps2", name="warm_ps")
        for wi in range(20):
            nc.tensor.matmul(
                wps[:, 0:D], warm[:, 0:P], warm[:],
                start=(wi == 0), stop=(wi == 19),
            )

        consts = ctx.enter_context(tc.tile_pool(name="consts", bufs=1))

        wq_sb, wk_sb, wv_sb, wp_sb = [], [], [], []
        for hi in range(2):
            for (lst, dram, nm) in (
                (wk_sb, wk_d, "wk"),
                (wv_sb, wv_d, "wv"),
                (wq_sb, wq_d, "wq"),
                (wp_sb, wp_d, "wp"),
            ):
                t = consts.tile([P, DC, D], fp8, tag=f"{nm}{hi}")
                nc.sync.dma_start(
                    t[:], dram[hi].rearrange("(c q) e -> q c e", q=P)
                )
                lst.append(t)

        bkvr_sb, bqe_sb, bqs_sb = [], [], []
        for hi in range(2):
            t = consts.tile([1, 2, 2 * D], fp8, tag=f"bkvr{hi}")
            nc.sync.dma_start(t[:], bkvr_d[hi])
            bkvr_sb.append(t)
            t = consts.tile([P, DC], f32, tag=f"bqe{hi}")
            nc.sync.dma_start(t[:], bqe_d[hi])
            bqe_sb.append(t)
            t = consts.tile([P, 3, DC], f32, tag=f"bqs{hi}")
            nc.sync.dma_start(t[:], bqs_d[hi])
            bqs_sb.append(t)

        # all-ones DR stationary/moving helper rows (1.0 each)
        onesdr = consts.tile([1, 2, D], fp8, tag="onesdr")
        nc.vector.memset(onesdr[:], 1.0)

        pe_sb = consts.tile([P, NC_, D], bf16, tag="pe")
        nc.sync.dma_start(pe_sb[:], pe_d.rearrange("(t q) e -> q t e", q=P))
        ones_sb = consts.tile([P, 2, 16], fp8, tag="ones")
        nc.vector.memset(ones_sb[:], WSCALE)
        lna64_sb = consts.tile([P, 1], f32, tag="lna64")
        nc.vector.memset(lna64_sb[:], math.log(ALPHA * WSCALE))

        def mm(out, lhsT, rhs, start, stop):
            nc.tensor.matmul(out, lhsT, rhs, start=start, stop=stop, perf_mode=DR)

        pair_tiles = {}

        def emit_A1(p, hi, xt):
            """qkv projections + selu' (k|v and q j0|j1 in 2-bank psum pairs)."""
            kv = kvpool.tile([P, NC_, 2 * D], fp8, tag="kv")
            for t in range(NC_):
                kp2 = ps2.tile([P, 2 * D], f32, tag="ps2", name="kp2")
                for g in range(DC // 2):
                    lhs = xt[:, 2 * g : 2 * g + 2, P * t : P * (t + 1)]
                    mm(kp2[:, 0:D], lhs, wk_sb[hi][:, 2 * g : 2 * g + 2, :],
                       g == 0, False)
                    mm(kp2[:, D : 2 * D], lhs, wv_sb[hi][:, 2 * g : 2 * g + 2, :],
                       g == 0, False)
                # bias as a K=1 DR accumulation: kp += ones^T @ [(bk|bv), 0]
                mm(kp2[:, 0:D], onesdr[:, :, 0:P], bkvr_sb[hi][:, :, 0:D],
                   False, True)
                mm(kp2[:, D : 2 * D], onesdr[:, :, 0:P],
                   bkvr_sb[hi][:, :, D : 2 * D], False, True)
                with tc.high_priority(offset=300):
                    ke = tb.tile([P, 2 * D], bf16, tag="tb")
                    nc.scalar.activation(
                        ke[:], kp2[:], AF.Exp, bias=lna64_sb[:],
                        scale=1.0 / WSCALE,
                    )
                    km = tb.tile([P, 2 * D], bf16, tag="tb")
                    nc.vector.tensor_scalar(
                        km[:], ke[:], -ALPHA * WSCALE, 0.0, ALU.add, ALU.min
                    )
                    if t in KV_RELU_SWAP_T:
                        # ACT evacuates max(U,0) (Relu shares Exp's table
                        # set) so the DVE combine is an SBUF-only 2x-mode
                        # tensor_tensor instead of a 1x PSUM read.
                        rk = tb.tile([P, 2 * D], bf16, tag="tb")
                        nc.scalar.activation(rk[:], kp2[:], AF.Relu)
                        nc.vector.tensor_tensor(
                            kv[:, t, :], rk[:], km[:], ALU.add
                        )
                    else:
                        nc.vector.scalar_tensor_tensor(
                            kv[:, t, :], kp2[:], 0.0, km[:], ALU.max, ALU.add
                        )

            # ---- q^T in [D, N] layout ----
            # Per-partition bias b rides the pointwise ops instead of a
            # matmul:  with U = qp + 64b,
            #   selu64(U) = max(U,0) + min(64a e^{U/64} - 64a, 0)
            #             = max(qp, -64b) + min(qe + (64b - 64a), 64b)
            # where qe = 64a e^{U/64} comes from the ACT bias port.
            qt = qtpool.tile([P, DC, N], fp8, tag="qt")
            for c in range(DC):
                qp2 = ps2.tile([P, N], f32, tag="ps2", name="qp2")
                for g in range(DC // 2):
                    lhs = wq_sb[hi][:, 2 * g : 2 * g + 2, P * c : P * (c + 1)]
                    for j in range(2):
                        mm(qp2[:, 512 * j : 512 * (j + 1)], lhs,
                           xt[:, 2 * g : 2 * g + 2, 512 * j : 512 * (j + 1)],
                           g == 0, g == DC // 2 - 1)
                with tc.high_priority(offset=300):
                    qe = tb.tile([P, N], bf16, tag="tb")
                    nc.scalar.activation(
                        qe[:], qp2[:], AF.Exp,
                        bias=bqe_sb[hi][:, c : c + 1], scale=1.0 / WSCALE,
                    )
                    qm = tb.tile([P, N], bf16, tag="tb")
                    nc.vector.tensor_scalar(
                        qm[:], qe[:],
                        bqs_sb[hi][:, 0, c : c + 1],
                        bqs_sb[hi][:, 1, c : c + 1],
                        ALU.add, ALU.min,
                    )
                    nc.vector.scalar_tensor_tensor(
                        qt[:, c, :], qp2[:],
                        bqs_sb[hi][:, 2, c : c + 1], qm[:],
                        ALU.max, ALU.add,
                    )

            return kv, qt

        def emit_A2(p, hi, kv, qt):
            """C = k'^T v' and exp(kappa L^T)."""
            # ---- C = k'^T v'  [D, D], c-chunk pairs in one 2-bank psum ----
            csb = cpool.tile([P, DC, D], fp8, tag="csb")
            for cp_ in range(DC // 2):
                cp2 = ps2.tile([P, 2 * D], f32, tag="ps2", name="cp2")
                for g in range(NC_ // 2):
                    for ci in range(2):
                        c = 2 * cp_ + ci
                        mm(cp2[:, 512 * ci : 512 * (ci + 1)],
                           kv[:, 2 * g : 2 * g + 2, P * c : P * (c + 1)],
                           kv[:, 2 * g : 2 * g + 2, D : 2 * D],
                           g == 0, g == NC_ // 2 - 1)
                with tc.high_priority(offset=300):
                    nc.scalar.mul(
                        csb[:, 2 * cp_ : 2 * cp_ + 2, :], cp2[:], 1.0 / CSC
                    )

            # ---- exp(kappa * L^T), L^T = C^T q^T  [D, N] ----
            elt = eltpool.tile([P, DC, N], fp8, tag="elt")
            for jc in range(DC):
                lp2 = ps2.tile([P, N], f32, tag="ps2", name="lp2")
                for g in range(DC // 2):
                    lhs = csb[:, 2 * g : 2 * g + 2, P * jc : P * (jc + 1)]
                    for j in range(2):
                        mm(lp2[:, 512 * j : 512 * (j + 1)], lhs,
                           qt[:, 2 * g : 2 * g + 2, 512 * j : 512 * (j + 1)],
                           g == 0, g == DC // 2 - 1)
                # qt is 64-scaled (bias rides the pointwise scalars), k/v
                # are 64-scaled each -> lp carries 64^3/CSC
                with tc.high_priority(offset=300):
                    nc.scalar.activation(
                        elt[:, jc, :], lp2[:], AF.Exp,
                        scale=KAPPA * CSC / (WSCALE * WSCALE * WSCALE),
                    )
            return elt

        def emit_B(p, hi, elt, tail=False):
            if hi == 0:
                pair_tiles[p] = zpool.tile(
                    [P, NC_, D], bf16, tag="zt", name=f"z_{p}"
                )
            zt = pair_tiles[p]
            osb = opool.tile([P, NC_, D], bf16, tag="osb", name=f"osb_{p}_{hi}") if hi == 1 else None
            rsr = rsrpool.tile([P, NC_], f32, tag="rsr", name=f"rsr_{p}_{hi}")
            out_r = out_d[p].rearrange("(t q) e -> q t e", q=P) if hi == 1 else None
            for t in range(NC_):
                pr2 = ps2.tile([P, 2 * D], f32, tag="ps2", name="pr2")
                pp = pr2[:, 0:D]
                rp = pr2[:, D : D + 1]
                for g in range(DC // 2):
                    lhs = elt[:, 2 * g : 2 * g + 2, P * t : P * (t + 1)]
                    mm(rp, lhs, ones_sb[:, :, 0:1], g == 0, g == DC // 2 - 1)
                    mm(pp, lhs, wp_sb[hi][:, 2 * g : 2 * g + 2, :],
                       g == 0, g == DC // 2 - 1)
                nc.vector.reciprocal(rsr[:, t : t + 1], rp)
                # bp is folded into Wp' on the host (softmax rows sum to 1)
                # so pre = pp * (1/rowsum) needs no tensor add: ACT Copy
                # with the per-partition reciprocal on the scale port.
                pre = tf.tile([P, D], bf16, tag="tf")
                nc.scalar.activation(
                    pre[:], pp, AF.Copy, bias=0.0, scale=rsr[:, t : t + 1]
                )
                # gelu(x) ~ x*(GC*x + 0.5) via Horner on the DVE.  At head 0
                # the eps factor of the final combine is folded into the
                # Horner coefficients.  The very last unit's chain splits
                # odd tiles onto the (otherwise idle) GPSIMD to shorten the
                # serial pipeline-drain tail.
                eng = nc.gpsimd if (tail and t % 2 == 1) else nc.vector
                ts1 = tf.tile([P, D], bf16, tag="tf")
                if hi == 0:
                    nc.vector.tensor_scalar(
                        ts1[:], pre[:], GC * EPS, 0.5 * EPS, ALU.mult, ALU.add
                    )
                else:
                    nc.vector.tensor_scalar(
                        ts1[:], pre[:], GC, 0.5, ALU.mult, ALU.add
                    )
                g0 = tf.tile([P, D], bf16, tag="tf")
                eng.tensor_tensor(g0[:], ts1[:], pre[:], ALU.mult)
                if hi == 0:
                    # z = eps*proj0 + pe, consumed by head 1's combine
                    eng.tensor_tensor(
                        zt[:, t, :], g0[:], pe_sb[:, t, :], ALU.add
                    )
                else:
                    eng.tensor_tensor(
                        osb[:, t, :], g0[:], zt[:, t, :], ALU.add
                    )
                    nc.sync.dma_start(out_r[:, t : t + 1, :], osb[:, t : t + 1, :])

        # two-deep software pipeline: emit A1[i] (kv+q matmuls), then
        # A2[i-1] (C+LT), then B[i-2] (rowsum/proj/combine).  Each stage's
        # inputs are a full unit old by the time its matmuls reach the PE
        # queue head, so the PE never waits on a same-unit pointwise chain.
        a2_pending = None   # (p, hi, kv, qt)
        b_pending = []      # [(p, hi, elt), ...]
        for p in range(n_pairs):
            if p == 0:
                xt = xt0
            else:
                xt = xpool.tile([P, DC, N], fp8, tag="xt")
                nc.sync.dma_start(xt[:], xT_d[p].rearrange("(c q) n -> q c n", q=P))
            for hi in range(2):
                kv, qt = emit_A1(p, hi, xt)
                if a2_pending is not None:
                    b_pending.append(
                        (a2_pending[0], a2_pending[1],
                         emit_A2(*a2_pending))
                    )
                if len(b_pending) > 1:
                    emit_B(*b_pending.pop(0))
                a2_pending = (p, hi, kv, qt)
        emit_B(*b_pending.pop(0))
        b_pending.append((a2_pending[0], a2_pending[1], emit_A2(*a2_pending)))
        emit_B(*b_pending.pop(0), tail=True)

    return nc


def _pose_encoding_table():
    idx = np.arange(N, dtype=np.float32)[:, None]
    ks = np.arange(D // 2, dtype=np.float32)[None, :]
    arg = idx / (1000.0 * (2.0 * ks / np.float32(D)) + np.float32(0.01))
    pe = np.zeros((N, D), np.float32)
    pe[:, 0::2] = np.sin(arg)
    pe[:, 1::2] = np.cos(arg)
    return pe


def _host_prep(x, Wqkv, bqkv, Wp, bp):
    x = np.asarray(x, np.float32)
    Wqkv = np.asarray(Wqkv, np.float32)
    bqkv = np.asarray(bqkv, np.float32)
    Wp = np.asarray(Wp, np.float32)
    bp = np.asarray(bp, np.float32)

    f8 = ml_dtypes.float8_e4m3
    xT = np.ascontiguousarray(
        x.reshape(B * S, N, D).transpose(0, 2, 1)
    ).astype(f8)  # [32, D, N]

    ws = np.float32(64.0)
    wq = np.stack([Wqkv[h][:, 0 * D : 1 * D] * ws for h in HEADS_USED]).astype(f8)
    wk = np.stack([Wqkv[h][:, 1 * D : 2 * D] * ws for h in HEADS_USED]).astype(f8)
    wv = np.stack([Wqkv[h][:, 2 * D : 3 * D] * ws for h in HEADS_USED]).astype(f8)
    # softmax rows sum to 1 -> the proj bias folds into the proj weights:
    # att @ (Wp + 1 bp^T) = att @ Wp + bp  exactly.
    wp = np.stack([(Wp[h] + bp[h][None, :]) * ws for h in HEADS_USED]).astype(f8)

    # k/v bias rows, DR-packed as (b*64, 0): [2, 1, 2, 2D]
    bkvr8 = np.zeros((2, 1, 2, 2 * D), np.float32)
    for i, h in enumerate(HEADS_USED):
        bkvr8[i, 0, 0, :] = bqkv[h][D : 3 * D] * 64.0
    bkvr8 = bkvr8.astype(f8)

    # q-branch per-partition bias columns: exp-port bias and the
    # tensor_scalar/stt scalar columns (64b-64a, 64b, -64b)
    bqe = np.zeros((2, P, DC), np.float32)
    bqs = np.zeros((2, P, 3, DC), np.float32)
    a64 = np.float32(ALPHA * 64.0)
    for i, h in enumerate(HEADS_USED):
        bcol = bqkv[h][:D].reshape(DC, P).T  # [P, DC]
        bqe[i] = bcol + np.float32(math.log(ALPHA * WSCALE))
        bqs[i, :, 0, :] = 64.0 * bcol - a64
        bqs[i, :, 1, :] = 64.0 * bcol
        bqs[i, :, 2, :] = -64.0 * bcol

    pe = _pose_encoding_table().astype(ml_dtypes.bfloat16)

    shared = {
        "wq": wq, "wk": wk, "wv": wv, "wp": wp,
        "bkvr8": bkvr8, "bqe": bqe, "bqs": bqs,
        "pe": pe,
    }
    in_maps = []
    for core in range(NCORES):
        m = dict(shared)
        m["xT"] = np.ascontiguousarray(xT[core * PAIRS : (core + 1) * PAIRS])
        in_maps.append(m)
    return in_maps


_prog_cache = {}


def _get_program():
    if "nc" not in _prog_cache:
        _prog_cache["nc"] = build_program()
    return _prog_cache["nc"]


def kernel(x, Wqkv, bqkv, Wp, bp, _trace=False):
    nc = _get_program()
    in_maps = _host_prep(x, Wqkv, bqkv, Wp, bp)
    res = run_bass_kernel_spmd(nc, in_maps, list(range(NCORES)), trace=_trace)
    full = np.empty((B * S, N, D), np.float32)
    for core in range(NCORES):
        full[core * PAIRS : (core + 1) * PAIRS] = np.asarray(
            res.results[core]["out"]
        ).astype(np.float32)
    out = full.reshape(B, S, N, D)
    if _trace:
        return out, res
    return out


# revision 27
# speedup vs baseline: 1.0385x; 1.0385x over previous
"""Trainium2 Bass kernel for nn_AttentionTest_14044543058050.

Reference computation (B=4, S=8, N=1024, D=512, HEADS=4):
    for h in heads:
        qkv = selu(x @ Wqkv[h] + bqkv[h]);  q,k,v = split(qkv)
        att = softmax((q @ k.T / D) @ v, axis=-1)      # softmax over D!
        proj_h = gelu(att @ Wp[h] + bp[h])
    out = pose_encoding(proj_3 + 0.01 * proj_0)

Key algebraic facts exploited (same as the 433us baseline):
  * heads 1 and 2 are dead code -> only heads 0 and 3 computed.
  * (q k^T) v reassociates exactly to q (k^T v) -- no N x N scores.
  * softmax(L) @ Wp defers normalization past the Wp matmul.
  * selu decomposed as max(U,0) + min(64a e^{U/64} - 64a, 0) with the
    lam^3/D constant folded into the exp(kappa L) activation scale.

v4 performance restructure over the 433us/377us baseline:
  * Every matmul accumulation pair (k|v, q j0|j1, C c|c+1, L j0|j1,
    proj|rowsum) now shares ONE two-bank PSUM tile.  Both half-groups
    become schedulable at the same sim time, so the Tile scheduler keeps
    the interleaved emission order and the post-legalize LDWEIGHTS dedup
    actually fires (the baseline's per-bank tiles made the scheduler
    serialize whole accumulation groups, separating same-key LDWs: only
    ~40 of ~500 dedup opportunities hit; now they all do).
  * All A1/A2 pointwise ops run once per 1024-wide pair instead of twice
    per 512: ACT exp (+ csb evacuate) amortize the (352cyc) fixed
    overhead; DVE ts/stt halve their per-op overhead and sem traffic.
  * softmax rows sum to 1, so the proj bias folds into the proj weights
    (Wp' = Wp + bp): with deferred normalization pre = pp*rsr where
    pp = sum(elt * 64(Wp+bp)) equals att@Wp + bp exactly.  The B-stage
    scalar_tensor_tensor disappears; pre is an ACT Copy with the
    per-partition 1/rowsum as the scale port.
  * GPSIMD runs (almost) NOTHING: its SBUF port is shared with the DVE,
    and hardware-measured here its tensor_tensor traffic degrades
    concurrent DVE tensor_scalar/tensor_tensor from their 2x/4x packed
    modes to ~1x (427ns -> 785ns, 409ns -> 1331ns).  The whole
    gelu/combine chain moved to the DVE, which is a net win once its
    fast modes survive; only the very last unit's drain (nothing else
    running) borrows GPSIMD for half its tiles.
  * The k/v selu uses the exact identity selu64(U) =
    min(max(U, 64u*), 64a e^{U/64} - 64a) with u* the root of
    a(e^u-1) = u, making the ke-shift a single-ALU tensor_scalar and
    putting the clamp on the stt scalar port.
  * Output, pose table and the epsilon-combine chain run in bf16; the
    f32 upcast happens on the host after the gather (DMA volume halves).
  The point of all of it: the PE (the roofline engine at ~200us of
  fp8-DoubleRow matmul stream) stops starving on the DVE semaphore and
  stops re-throttling to K=4/8 (the baseline oscillated HAM 7 times,
  ~110us at half clock).

Sharding: the 32 (b, s) pairs are split 4-per-core across 8 NeuronCores;
weights are replicated.  Both live heads of a pair stay on one core.

Precision: all matmuls fp8e4m3 DoubleRow (weights pre-scaled by 64,
scales cancel inside exp(kappa L) as in the baseline).  bf16 output
rounding adds ~4e-3 of output absmax; total measured error stays well
inside the 2e-2 gate.
"""

import math
from contextlib import ExitStack

import numpy as np
import ml_dtypes

import concourse.bass as bass
import concourse.tile as tile
import concourse.mybir as mybir
from concourse.vector_clock import ScopedClock
from concourse.bass_utils import run_bass_kernel_spmd

B, S, N, D = 4, 8, 1024, 512
HEADS_USED = (0, 3)
EPS = 0.01
LAM = 1.0507009873554805
ALPHA = 1.6732632423543772
USTAR = -1.1359502408965148  # root of alpha*(e^u - 1) = u;  selu64(U) =
# min(max(U, 64*u*), 64a*e^{U/64} - 64a) exactly (verified to 0 ulp on a grid)
LN_ALPHA = math.log(ALPHA)
KAPPA = LAM ** 3 / D
NCORES = 8
PAIRS = (B * S) // NCORES  # 4 (b,s) pairs per core

bf16 = mybir.dt.bfloat16
f32 = mybir.dt.float32
fp8 = mybir.dt.float8e4
DR = mybir.MatmulPerfMode.DoubleRow
WSCALE = 64.0
CSC = 2048.0  # C-cast divisor: keeps |csb| < fp8e4m3 max 240
AF = mybir.ActivationFunctionType
ALU = mybir.AluOpType
P = 128
DC = D // P   # 4 chunks of 128 along D
NC_ = N // P  # 8 chunks of 128 along N
GC = 0.3989422804014327  # gelu(x) ~ x*(GC*x + 0.5), exact to <2e-6 here
# ACT-relu offload of k/v selu tiles: measured a wash (the fp8-output
# tensor_tensor runs at 1x, same as the PSUM-read stt it replaces) while
# costing an extra ACT pass -- disabled.
KV_RELU_SWAP_T = ()


class _SplitDrainTileContext(tile.TileContext):
    """TileContext adapted to this container's walrus build, which rejects
    more than ONE sync-wait command per instruction (any format).  After
    Tile assigns semaphores we hoist every extra wait onto a same-engine
    NoOp inserted right before the instruction (engine queues are in-order,
    so waiting earlier on the same queue is equivalent), and the final
    drain's aggregated wait list is split the same way.

    Additionally dedupes LDWEIGHTS: tile_legalize splits every non-f32
    matmul into InstLdweights + non-self-loading InstMatmult, reloading
    the stationary operand even when consecutive matmuls share it.  The
    PE serializes LDWEIGHTS with the matmul stream (~140 ns each), so a
    repeated load of the identical weights AP is pure loss.  We replace
    any InstLdweights whose (offset, access pattern, dtype, perf mode)
    equals the previous weight load on the PE queue -- with no other
    weight-mutating PE instruction in between -- by a NoOp carrying the
    same sync_info (waits and sem updates preserved)."""

    def _dedupe_ldweights(self):
        nc = self.nc
        pe = mybir.EngineType.PE
        deduped = 0
        for f in nc.m.functions:
            for bb in f.blocks:
                last_key = None
                newl = []
                for inst in bb.instructions:
                    tn = type(inst).__name__
                    if tn == "InstLdweights":
                        key = None
                        try:
                            arg = inst.ins[0]
                            key = (
                                arg.offset,
                                tuple(tuple(p) for p in arg.ap),
                                str(arg.dtype),
                                str(inst.perf_mode),
                                inst.is_transpose,
                                inst.tile_size,
                                inst.tile_position,
                            )
                        except Exception:
                            key = None
                        if key is not None and key == last_key:
                            nop = mybir.InstNoOp(
                                name=nc.get_next_instruction_name(),
                                ins=[], outs=[],
                            )
                            nop.engine = inst.engine
                            nop.sync_info = inst.sync_info
                            nc.register_instruction(nop)
                            newl.append(nop)
                            deduped += 1
                            continue
                        last_key = key
                    elif inst.engine == pe:
                        if tn == "InstMatmult":
                            if inst.ldweights is not False:
                                # self-loading matmul clobbers the array
                                last_key = None
                        elif tn not in (
                            "InstNoOp", "InstEventSemaphore", "InstDrain"
                        ):
                            last_key = None
                    newl.append(inst)
                bb.instructions = newl
        self._ldw_deduped = deduped

    def _hoist_extra_waits(self):
        nc = self.nc
        for f in nc.m.functions:
            for bb in f.blocks:
                insts = bb.instructions
                if not any(
                    i.sync_info and i.sync_info.on_wait and len(i.sync_info.on_wait) > 1
                    for i in insts
                ):
                    continue
                newl = []
                for inst in insts:
                    si = inst.sync_info
                    if si and si.on_wait and len(si.on_wait) > 1:
                        waits = list(si.on_wait)
                        for w in waits[:-1]:
                            nop = mybir.InstNoOp(
                                name=nc.get_next_instruction_name(), ins=[], outs=[]
                            )
                            nop.engine = inst.engine
                            nop.sync_info = mybir.SyncInfo(
                                on_wait=[w], on_update=[]
                            )
                            nc.register_instruction(nop)
                            newl.append(nop)
                        si.on_wait = [waits[-1]]
                    newl.append(inst)
                bb.instructions = newl

    def _drain_and_barrier(self, tick_clock, wait_clock):
        nc = self.nc
        self._dedupe_ldweights()
        self._hoist_extra_waits()
        nop0 = nc.sync.nop(nofuse=True)
        wait_clock.add_sem_waits(
            nop0.ins, ScopedClock({None: tick_clock.global_clock})
        )
        si = nop0.ins.sync_info
        waits = list(si.on_wait) if si is not None and si.on_wait else []
        if len(waits) > 1:
            si.on_wait = waits[:1]
            for w in waits[1:]:
                nop = nc.sync.nop(nofuse=True)
                nsi = nop.ins.sync_info
                if nsi is None:
                    nop.ins.sync_info = mybir.SyncInfo(on_wait=[w], on_update=[])
                else:
                    nsi.on_wait = [w]
        nc.sync.drain()
        nc.all_engine_barrier()
        assert self.sems is not None
        popped = nc._tile_sem_poison_stack.pop()
        assert popped is self._sem_poison
        nc.clear_and_free_semaphores(list(self.sems.allocated().values()))
        nc.all_engine_barrier()


def build_program(n_pairs=PAIRS):
    nc = bass.Bass()

    xT_d = nc.dram_tensor("xT", [n_pairs, D, N], fp8, kind="ExternalInput")
    wq_d = nc.dram_tensor("wq", [2, D, D], fp8, kind="ExternalInput")
    wk_d = nc.dram_tensor("wk", [2, D, D], fp8, kind="ExternalInput")
    wv_d = nc.dram_tensor("wv", [2, D, D], fp8, kind="ExternalInput")
    wp_d = nc.dram_tensor("wp", [2, D, D], fp8, kind="ExternalInput")
    bkvr_d = nc.dram_tensor("bkvr8", [2, 1, 2, 2 * D], fp8, kind="ExternalInput")
    bqe_d = nc.dram_tensor("bqe", [2, P, DC], f32, kind="ExternalInput")
    bqs_d = nc.dram_tensor("bqs", [2, P, 3, DC], f32, kind="ExternalInput")
    pe_d = nc.dram_tensor("pe", [N, D], bf16, kind="ExternalInput")
    out_d = nc.dram_tensor("out", [n_pairs, N, D], bf16, kind="ExternalOutput")

    with _SplitDrainTileContext(nc) as tc, ExitStack() as ctx:
        xpool = ctx.enter_context(tc.tile_pool(name="xt", bufs=2))
        qtpool = ctx.enter_context(tc.tile_pool(name="qt", bufs=2))
        kvpool = ctx.enter_context(tc.tile_pool(name="kv", bufs=2))
        cpool = ctx.enter_context(tc.tile_pool(name="csb", bufs=2))
        eltpool = ctx.enter_context(tc.tile_pool(name="elt", bufs=4))
        zpool = ctx.enter_context(tc.tile_pool(name="zt", bufs=2))
        opool = ctx.enter_context(tc.tile_pool(name="osb", bufs=2))
        rsrpool = ctx.enter_context(tc.tile_pool(name="rsr", bufs=3))
        tb = ctx.enter_context(tc.tile_pool(name="tb", bufs=8))
        tf = ctx.enter_context(tc.tile_pool(name="tf", bufs=8))
        # single two-bank-per-tile PSUM pool: every accumulation pair
        # (k|v, q j0|j1, C c|c+1, L j0|j1, proj|rowsum) lives in one tile
        # so both half-groups allocate (and become schedulable) together.
        ps2 = ctx.enter_context(tc.tile_pool(name="ps2", bufs=4, space="PSUM"))

        xt0 = xpool.tile([P, DC, N], fp8, tag="xt", name="xt_pre0")
        nc.sync.dma_start(xt0[:], xT_d[0].rearrange("(c q) n -> q c n", q=P))

        wpool = ctx.enter_context(tc.tile_pool(name="warm", bufs=1))
        warm = wpool.tile([P, 512], bf16, tag="warm")
        nc.vector.memset(warm[:], 0.0)
        wps = ps2.tile([P, 2 * D], f32, tag="ps2", name="warm_ps")
        for wi in range(26):
            nc.tensor.matmul(
                wps[:, 0:D], warm[:, 0:P], warm[:],
                start=(wi == 0), stop=(wi == 25),
            )

        consts = ctx.enter_context(tc.tile_pool(name="consts", bufs=1))

        # head-0 weights first (they gate the first A1 matmuls), then
        # head-1 and the pose table (first needed a unit later).
        wq_sb, wk_sb, wv_sb, wp_sb = [], [], [], []
        for hi in range(2):
            for (lst, dram, nm) in (
                (wk_sb, wk_d, "wk"),
                (wv_sb, wv_d, "wv"),
                (wq_sb, wq_d, "wq"),
                (wp_sb, wp_d, "wp"),
            ):
                t = consts.tile([P, DC, D], fp8, tag=f"{nm}{hi}")
                lst.append(t)
        for hi in range(2):
            for (lst, dram) in (
                (wk_sb, wk_d), (wv_sb, wv_d), (wq_sb, wq_d), (wp_sb, wp_d),
            ):
                nc.sync.dma_start(
                    lst[hi][:], dram[hi].rearrange("(c q) e -> q c e", q=P)
                )

        bkvr_sb, bqe_sb, bqs_sb = [], [], []
        for hi in range(2):
            t = consts.tile([1, 2, 2 * D], fp8, tag=f"bkvr{hi}")
            nc.sync.dma_start(t[:], bkvr_d[hi])
            bkvr_sb.append(t)
            t = consts.tile([P, DC], f32, tag=f"bqe{hi}")
            nc.sync.dma_start(t[:], bqe_d[hi])
            bqe_sb.append(t)
            t = consts.tile([P, 3, DC], f32, tag=f"bqs{hi}")
            nc.sync.dma_start(t[:], bqs_d[hi])
            bqs_sb.append(t)

        # all-ones DR stationary/moving helper rows (1.0 each)
        onesdr = consts.tile([1, 2, D], fp8, tag="onesdr")
        nc.vector.memset(onesdr[:], 1.0)

        pe_sb = consts.tile([P, NC_, D], bf16, tag="pe")
        nc.sync.dma_start(pe_sb[:], pe_d.rearrange("(t q) e -> q t e", q=P))
        ones_sb = consts.tile([P, 2, 16], fp8, tag="ones")
        nc.vector.memset(ones_sb[:], WSCALE)
        lna64_sb = consts.tile([P, 1], f32, tag="lna64")
        nc.vector.memset(lna64_sb[:], math.log(ALPHA * WSCALE))

        def mm(out, lhsT, rhs, start, stop):
            nc.tensor.matmul(out, lhsT, rhs, start=start, stop=stop, perf_mode=DR)

        pair_tiles = {}

        def emit_A1(p, hi, xt):
            """qkv projections + selu' (k|v and q j0|j1 in 2-bank psum pairs)."""
            kv = kvpool.tile([P, NC_, 2 * D], fp8, tag="kv")
            for t in range(NC_):
                kp2 = ps2.tile([P, 2 * D], f32, tag="ps2", name="kp2")
                for g in range(DC // 2):
                    lhs = xt[:, 2 * g : 2 * g + 2, P * t : P * (t + 1)]
                    mm(kp2[:, 0:D], lhs, wk_sb[hi][:, 2 * g : 2 * g + 2, :],
                       g == 0, False)
                    mm(kp2[:, D : 2 * D], lhs, wv_sb[hi][:, 2 * g : 2 * g + 2, :],
                       g == 0, False)
                # bias as a K=1 DR accumulation: kp += ones^T @ [(bk|bv), 0]
                mm(kp2[:, 0:D], onesdr[:, :, 0:P], bkvr_sb[hi][:, :, 0:D],
                   False, True)
                mm(kp2[:, D : 2 * D], onesdr[:, :, 0:P],
                   bkvr_sb[hi][:, :, D : 2 * D], False, True)
                with tc.high_priority(offset=300):
                    ke = tb.tile([P, 2 * D], bf16, tag="tb")
                    nc.scalar.activation(
                        ke[:], kp2[:], AF.Exp, bias=lna64_sb[:],
                        scale=1.0 / WSCALE,
                    )
                    # selu64(U) = min(max(U, 64u*), ke - 64a) exactly
                    # (u* the root of a(e^u-1)=u); the ke shift is a
                    # SINGLE-alu tensor_scalar, the clamp rides the stt
                    # scalar port.
                    km = tb.tile([P, 2 * D], bf16, tag="tb")
                    nc.vector.tensor_scalar(
                        km[:], ke[:], -ALPHA * WSCALE, 0.0, ALU.add, ALU.bypass
                    )
                    nc.vector.scalar_tensor_tensor(
                        kv[:, t, :], kp2[:], USTAR * WSCALE, km[:],
                        ALU.max, ALU.min,
                    )

            # ---- q^T in [D, N] layout ----
            # Per-partition bias b rides the pointwise ops instead of a
            # matmul:  with U = qp + 64b,
            #   selu64(U) = max(U,0) + min(64a e^{U/64} - 64a, 0)
            #             = max(qp, -64b) + min(qe + (64b - 64a), 64b)
            # where qe = 64a e^{U/64} comes from the ACT bias port.
            qt = qtpool.tile([P, DC, N], fp8, tag="qt")
            for c in range(DC):
                qp2 = ps2.tile([P, N], f32, tag="ps2", name="qp2")
                for g in range(DC // 2):
                    lhs = wq_sb[hi][:, 2 * g : 2 * g + 2, P * c : P * (c + 1)]
                    for j in range(2):
                        mm(qp2[:, 512 * j : 512 * (j + 1)], lhs,
                           xt[:, 2 * g : 2 * g + 2, 512 * j : 512 * (j + 1)],
                           g == 0, g == DC // 2 - 1)
                with tc.high_priority(offset=300):
                    qe = tb.tile([P, N], bf16, tag="tb")
                    nc.scalar.activation(
                        qe[:], qp2[:], AF.Exp,
                        bias=bqe_sb[hi][:, c : c + 1], scale=1.0 / WSCALE,
                    )
                    qm = tb.tile([P, N], bf16, tag="tb")
                    nc.vector.tensor_scalar(
                        qm[:], qe[:],
                        bqs_sb[hi][:, 0, c : c + 1],
                        bqs_sb[hi][:, 1, c : c + 1],
                        ALU.add, ALU.min,
                    )
                    nc.vector.scalar_tensor_tensor(
                        qt[:, c, :], qp2[:],
                        bqs_sb[hi][:, 2, c : c + 1], qm[:],
                        ALU.max, ALU.add,
                    )

            return kv, qt

        def emit_A2(p, hi, kv, qt):
            """C = k'^T v' and exp(kappa L^T)."""
            # ---- C = k'^T v'  [D, D], c-chunk pairs in one 2-bank psum ----
            csb = cpool.tile([P, DC, D], fp8, tag="csb")
            for cp_ in range(DC // 2):
                cp2 = ps2.tile([P, 2 * D], f32, tag="ps2", name="cp2")
                for g in range(NC_ // 2):
                    for ci in range(2):
                        c = 2 * cp_ + ci
                        mm(cp2[:, 512 * ci : 512 * (ci + 1)],
                           kv[:, 2 * g : 2 * g + 2, P * c : P * (c + 1)],
                           kv[:, 2 * g : 2 * g + 2, D : 2 * D],
                           g == 0, g == NC_ // 2 - 1)
                with tc.high_priority(offset=300):
                    nc.scalar.mul(
                        csb[:, 2 * cp_ : 2 * cp_ + 2, :], cp2[:], 1.0 / CSC
                    )

            # ---- exp(kappa * L^T), L^T = C^T q^T  [D, N] ----
            elt = eltpool.tile([P, DC, N], fp8, tag="elt")
            for jc in range(DC):
                lp2 = ps2.tile([P, N], f32, tag="ps2", name="lp2")
                for g in range(DC // 2):
                    lhs = csb[:, 2 * g : 2 * g + 2, P * jc : P * (jc + 1)]
                    for j in range(2):
                        mm(lp2[:, 512 * j : 512 * (j + 1)], lhs,
                           qt[:, 2 * g : 2 * g + 2, 512 * j : 512 * (j + 1)],
                           g == 0, g == DC // 2 - 1)
                # qt is 64-scaled (bias rides the pointwise scalars), k/v
                # are 64-scaled each -> lp carries 64^3/CSC
                with tc.high_priority(offset=300):
                    nc.scalar.activation(
                        elt[:, jc, :], lp2[:], AF.Exp,
                        scale=KAPPA * CSC / (WSCALE * WSCALE * WSCALE),
                    )
            return elt

        def emit_B(p, hi, elt, tail=False):
            if hi == 0:
                pair_tiles[p] = zpool.tile(
                    [P, NC_, D], bf16, tag="zt", name=f"z_{p}"
                )
            zt = pair_tiles[p]
            osb = opool.tile([P, NC_, D], bf16, tag="osb", name=f"osb_{p}_{hi}") if hi == 1 else None
            rsr = rsrpool.tile([P, NC_], f32, tag="rsr", name=f"rsr_{p}_{hi}")
            out_r = out_d[p].rearrange("(t q) e -> q t e", q=P) if hi == 1 else None
            for t in range(NC_):
                pr2 = ps2.tile([P, 2 * D], f32, tag="ps2", name="pr2")
                pp = pr2[:, 0:D]
                rp = pr2[:, D : D + 1]
                for g in range(DC // 2):
                    lhs = elt[:, 2 * g : 2 * g + 2, P * t : P * (t + 1)]
                    mm(rp, lhs, ones_sb[:, :, 0:1], g == 0, g == DC // 2 - 1)
                    mm(pp, lhs, wp_sb[hi][:, 2 * g : 2 * g + 2, :],
                       g == 0, g == DC // 2 - 1)
                nc.vector.reciprocal(rsr[:, t : t + 1], rp)
                # bp is folded into Wp' on the host (softmax rows sum to 1)
                # so pre = pp * (1/rowsum) needs no tensor add: ACT Copy
                # with the per-partition reciprocal on the scale port.
                pre = tf.tile([P, D], bf16, tag="tf")
                nc.scalar.activation(
                    pre[:], pp, AF.Copy, bias=0.0, scale=rsr[:, t : t + 1]
                )
                # gelu(x) ~ x*(GC*x + 0.5) via Horner on the DVE.  At head 0
                # the eps factor of the final combine is folded into the
                # Horner coefficients.  The very last unit's chain splits
                # odd tiles onto the (otherwise idle) GPSIMD to shorten the
                # serial pipeline-drain tail.
                eng = nc.gpsimd if (tail and t % 2 == 1) else nc.vector
                ts1 = tf.tile([P, D], bf16, tag="tf")
                if hi == 0:
                    nc.vector.tensor_scalar(
                        ts1[:], pre[:], GC * EPS, 0.5 * EPS, ALU.mult, ALU.add
                    )
                else:
                    nc.vector.tensor_scalar(
                        ts1[:], pre[:], GC, 0.5, ALU.mult, ALU.add
                    )
                g0 = tf.tile([P, D], bf16, tag="tf")
                eng.tensor_tensor(g0[:], ts1[:], pre[:], ALU.mult)
                if hi == 0:
                    # z = eps*proj0 + pe, consumed by head 1's combine
                    eng.tensor_tensor(
                        zt[:, t, :], g0[:], pe_sb[:, t, :], ALU.add
                    )
                else:
                    eng.tensor_tensor(
                        osb[:, t, :], g0[:], zt[:, t, :], ALU.add
                    )
            if hi == 1:
                nc.sync.dma_start(out_r[:], osb[:])

        # two-deep software pipeline: emit A1[i] (kv+q matmuls), then
        # A2[i-1] (C+LT), then B[i-2] (rowsum/proj/combine).  Each stage's
        # inputs are a full unit old by the time its matmuls reach the PE
        # queue head, so the PE never waits on a same-unit pointwise chain.
        a2_pending = None   # (p, hi, kv, qt)
        b_pending = []      # [(p, hi, elt), ...]
        for p in range(n_pairs):
            if p == 0:
                xt = xt0
            else:
                xt = xpool.tile([P, DC, N], fp8, tag="xt")
                nc.sync.dma_start(xt[:], xT_d[p].rearrange("(c q) n -> q c n", q=P))
            for hi in range(2):
                kv, qt = emit_A1(p, hi, xt)
                if a2_pending is not None:
                    b_pending.append(
                        (a2_pending[0], a2_pending[1],
                         emit_A2(*a2_pending))
                    )
                if len(b_pending) > 2:
                    emit_B(*b_pending.pop(0))
                a2_pending = (p, hi, kv, qt)
        emit_B(*b_pending.pop(0))
        b_pending.append((a2_pending[0], a2_pending[1], emit_A2(*a2_pending)))
        emit_B(*b_pending.pop(0))
        emit_B(*b_pending.pop(0), tail=True)

    return nc


def _pose_encoding_table():
    idx = np.arange(N, dtype=np.float32)[:, None]
    ks = np.arange(D // 2, dtype=np.float32)[None, :]
    arg = idx / (1000.0 * (2.0 * ks / np.float32(D)) + np.float32(0.01))
    pe = np.zeros((N, D), np.float32)
    pe[:, 0::2] = np.sin(arg)
    pe[:, 1::2] = np.cos(arg)
    return pe


def _host_prep(x, Wqkv, bqkv, Wp, bp):
    x = np.asarray(x, np.float32)
    Wqkv = np.asarray(Wqkv, np.float32)
    bqkv = np.asarray(bqkv, np.float32)
    Wp = np.asarray(Wp, np.float32)
    bp = np.asarray(bp, np.float32)

    f8 = ml_dtypes.float8_e4m3
    xT = np.ascontiguousarray(
        x.reshape(B * S, N, D).transpose(0, 2, 1)
    ).astype(f8)  # [32, D, N]

    ws = np.float32(64.0)
    wq = np.stack([Wqkv[h][:, 0 * D : 1 * D] * ws for h in HEADS_USED]).astype(f8)
    wk = np.stack([Wqkv[h][:, 1 * D : 2 * D] * ws for h in HEADS_USED]).astype(f8)
    wv = np.stack([Wqkv[h][:, 2 * D : 3 * D] * ws for h in HEADS_USED]).astype(f8)
    # softmax rows sum to 1 -> the proj bias folds into the proj weights:
    # att @ (Wp + 1 bp^T) = att @ Wp + bp  exactly.
    wp = np.stack([(Wp[h] + bp[h][None, :]) * ws for h in HEADS_USED]).astype(f8)

    # k/v bias rows, DR-packed as (b*64, 0): [2, 1, 2, 2D]
    bkvr8 = np.zeros((2, 1, 2, 2 * D), np.float32)
    for i, h in enumerate(HEADS_USED):
        bkvr8[i, 0, 0, :] = bqkv[h][D : 3 * D] * 64.0
    bkvr8 = bkvr8.astype(f8)

    # q-branch per-partition bias columns: exp-port bias and the
    # tensor_scalar/stt scalar columns (64b-64a, 64b, -64b)
    bqe = np.zeros((2, P, DC), np.float32)
    bqs = np.zeros((2, P, 3, DC), np.float32)
    a64 = np.float32(ALPHA * 64.0)
    for i, h in enumerate(HEADS_USED):
        bcol = bqkv[h][:D].reshape(DC, P).T  # [P, DC]
        bqe[i] = bcol + np.float32(math.log(ALPHA * WSCALE))
        bqs[i, :, 0, :] = 64.0 * bcol - a64
        bqs[i, :, 1, :] = 64.0 * bcol
        bqs[i, :, 2, :] = -64.0 * bcol

    pe = _pose_encoding_table().astype(ml_dtypes.bfloat16)

    shared = {
        "wq": wq, "wk": wk, "wv": wv, "wp": wp,
        "bkvr8": bkvr8, "bqe": bqe, "bqs": bqs,
        "pe": pe,
    }
    in_maps = []
    for core in range(NCORES):
        m = dict(shared)
        m["xT"] = np.ascontiguousarray(xT[core * PAIRS : (core + 1) * PAIRS])
        in_maps.append(m)
    return in_maps


_prog_cache = {}


def _get_program():
    if "nc" not in _prog_cache:
        _prog_cache["nc"] = build_program()
    return _prog_cache["nc"]


def kernel(x, Wqkv, bqkv, Wp, bp, _trace=False):
    nc = _get_program()
    in_maps = _host_prep(x, Wqkv, bqkv, Wp, bp)
    res = run_bass_kernel_spmd(nc, in_maps, list(range(NCORES)), trace=_trace)
    full = np.empty((B * S, N, D), np.float32)
    for core in range(NCORES):
        full[core * PAIRS : (core + 1) * PAIRS] = np.asarray(
            res.results[core]["out"]
        ).astype(np.float32)
    out = full.reshape(B, S, N, D)
    if _trace:
        return out, res
    return out
